# revision 11
# baseline (speedup 1.0000x reference)
"""Distributed Bass kernel for nn_AttentionLayer (B=2, Q=2048, KV=2048, D=1024, H=16).

Sharding: 8 cores = 2 batches x 4 head-groups (4 heads / 256 dims each).
Per core:
  qp^T = (Wq^T/8 slice) @ q^T       (bias folded, 1/sqrt(64) folded)
  kcat^T = [past_k^T slice | kp^T]  (kp^T = Wk^T slice @ k^T)
  v_aug  = normal-layout v_cat slice with a ones column per head
  S^T(kv,q) = kh^T.T @ qh^T ; P^T = exp(S^T) * (1-mask)^T
  out^T(65,q) = v_aug.T @ P^T       (row 64 = softmax denominators)
  x^T partial = Wl^T slice.T @ out_all^T  -> ReduceScatter(group of 4) -> +bl
Host does layout prep (transpose/cast/scale) and final gather/assembly.
"""

import os
import sys
import numpy as np

for _p in ("/opt/trn_rl_repo",):
    if _p not in sys.path:
        sys.path.insert(0, _p)

import ml_dtypes

BF16 = ml_dtypes.bfloat16

# ---- problem constants (hardcoded per spec) ----
B, Q, KV, D, H = 2, 2048, 2048, 1024, 16
HD = D // H           # 64
N_CORES = 8
GROUPS = 4            # head groups (= cores per batch)
HPC = H // GROUPS     # heads per core = 4
C = HPC * HD          # head-cols per core = 256
KVT = 2 * KV          # concat kv length = 4096
P = 128               # partitions
NT = 512              # matmul free-dim tile


def build_graph(q_len=Q, kv_len=KV, d=D, heads_per_core=HPC):
    """Build the SPMD Bass graph (same program on all 8 cores)."""
    import concourse.bass as bass
    import concourse.bacc as bacc
    import concourse.tile as tile
    from concourse import mybir

    dt = mybir.dt
    Act = mybir.ActivationFunctionType
    Alu = mybir.AluOpType

    c = heads_per_core * HD              # 256
    kvt = 2 * kv_len
    n_qt = q_len // NT                   # q free-dim tiles (4)
    n_kvc = kvt // P                     # kv partition chunks (32)
    n_dc = d // P                        # D chunks (8)
    n_cc = c // P                        # head-col chunks (2)

    nc = bacc.Bacc("TRN2", target_bir_lowering=False, debug=False,
                   num_devices=N_CORES)

    # ---- I/O ----
    qT = nc.dram_tensor("qT", [d, q_len], dt.bfloat16, kind="ExternalInput")
    kT = nc.dram_tensor("kT", [d, kv_len], dt.bfloat16, kind="ExternalInput")
    vT = nc.dram_tensor("vT", [d, kv_len], dt.bfloat16, kind="ExternalInput")
    pkT = nc.dram_tensor("pkT", [c, kv_len], dt.bfloat16, kind="ExternalInput")
    pv = nc.dram_tensor("pv", [kv_len, c], dt.bfloat16, kind="ExternalInput")
    maskt = nc.dram_tensor("maskt", [n_qt, n_kvc, P, NT], dt.bfloat16,
                           kind="ExternalInput")
    wqT = nc.dram_tensor("wqT", [d, c], dt.bfloat16, kind="ExternalInput")
    wkT = nc.dram_tensor("wkT", [d, c], dt.bfloat16, kind="ExternalInput")
    wvT = nc.dram_tensor("wvT", [d, c], dt.bfloat16, kind="ExternalInput")
    wlT = nc.dram_tensor("wlT", [c, d], dt.bfloat16, kind="ExternalInput")
    bq = nc.dram_tensor("bq", [c, 1], dt.float32, kind="ExternalInput")
    bk = nc.dram_tensor("bk", [c, 1], dt.float32, kind="ExternalInput")
    bv = nc.dram_tensor("bv", [c, 1], dt.float32, kind="ExternalInput")
    bl = nc.dram_tensor("bl", [d // GROUPS, 1], dt.float32, kind="ExternalInput")
    ident = nc.dram_tensor("ident", [P, P], dt.bfloat16, kind="ExternalInput")

    kpT_out = nc.dram_tensor("kpT", [c, kv_len], dt.bfloat16, kind="ExternalOutput")
    vpT_out = nc.dram_tensor("vpT", [c, kv_len], dt.bfloat16, kind="ExternalOutput")
    xT_out = nc.dram_tensor("xT", [d // GROUPS, q_len], dt.float32,
                            kind="ExternalOutput")

    rg = [[0, 1, 2, 3], [4, 5, 6, 7]]

    with tile.TileContext(nc) as tc:
        with (
            tc.tile_pool(name="const", bufs=1) as constp,
            tc.tile_pool(name="xt", bufs=9) as xtp,
            tc.tile_pool(name="proj", bufs=1) as projp,
            tc.tile_pool(name="vaug", bufs=1) as vaugp,
            tc.tile_pool(name="mask", bufs=n_kvc + 4) as maskp,
            tc.tile_pool(name="attn", bufs=3) as attnp,
            tc.tile_pool(name="psum", bufs=3, space="PSUM") as psump,
            tc.tile_pool(name="psum_o", bufs=2, space="PSUM") as psumop,
            tc.tile_pool(name="psum_t", bufs=2, space="PSUM") as psumtp,
            tc.tile_pool(name="dram", bufs=1, space="DRAM") as dramp,
        ):
            # ---- constants ----
            idn = constp.tile([P, P], dt.bfloat16, tag="ident", name="ident_sb")
            nc.sync.dma_start(idn[:], ident[:])
            ones_hd = constp.tile([1, HD], dt.float32, tag="ones", name="ones_hd")
            nc.vector.memset(ones_hd[:], 1.0)
            w_sb = {}
            for name, w in (("wq", wqT), ("wk", wkT), ("wv", wvT)):
                tiles = []
                for i in range(n_dc):
                    t = constp.tile([P, c], dt.bfloat16, tag=f"{name}{i}", name=f"{name}{i}")
                    nc.sync.dma_start(t[:], w[i * P:(i + 1) * P, :])
                    tiles.append(t)
                w_sb[name] = tiles
            wl_sb = []
            for i in range(n_cc):
                t = constp.tile([P, d], dt.bfloat16, tag=f"wl{i}", name=f"wl{i}")
                nc.sync.dma_start(t[:], wlT[i * P:(i + 1) * P, :])
                wl_sb.append(t)
            b_sb = {}
            for name, bten in (("bq", bq), ("bk", bk), ("bv", bv), ("bl", bl)):
                tiles = []
                for i in range(n_cc):
                    t = constp.tile([P, 1], dt.float32, tag=f"{name}{i}", name=f"{name}{i}")
                    nc.sync.dma_start(t[:], bten[i * P:(i + 1) * P, :])
                    tiles.append(t)
                b_sb[name] = tiles

            # ---- persistent activations ----
            qp_t = [constp.tile([P, q_len], dt.bfloat16, tag=f"qpt{g}", name=f"qpt{g}")
                    for g in range(n_cc)]
            kcat_t = [constp.tile([P, kvt], dt.bfloat16, tag=f"kct{g}", name=f"kct{g}")
                      for g in range(n_cc)]
            out_t = [constp.tile([P, q_len], dt.bfloat16, tag=f"ot{g}", name=f"ot{g}")
                     for g in range(n_cc)]
            vaug = [vaugp.tile([P, heads_per_core * (HD + 1)], dt.bfloat16,
                               tag=f"va{i}", name=f"va{i}") for i in range(n_kvc)]

            # past_k^T -> left half of kcat_t
            for g in range(n_cc):
                nc.sync.dma_start(kcat_t[g][:, 0:kv_len], pkT[g * P:(g + 1) * P, :])
            # past_v -> vaug chunks [0 .. n_kvc/2), strided head groups + ones col
            for i in range(n_kvc // 2):
                va = vaug[i].rearrange("p (g x) -> p g x", x=HD + 1)
                src = pv[i * P:(i + 1) * P, :].rearrange("p (g x) -> p g x", x=HD)
                nc.sync.dma_start(va[:, :, 0:HD], src)
                nc.vector.memset(va[:, :, HD:HD + 1], 1.0)
            for i in range(n_kvc // 2, n_kvc):
                va = vaug[i].rearrange("p (g x) -> p g x", x=HD + 1)
                nc.vector.memset(va[:, :, HD:HD + 1], 1.0)

            # ---- projections ----
            def project(xT_dram, n_cols, w_tiles, bias_tiles, sink):
                """sink(g, t, psum_ap) consumes each (128, NT) projected tile
                of (W^T slice @ x^T) = (c x n_cols), in ^T layout."""
                xs = [xtp.tile([P, n_cols], dt.bfloat16, tag="xt", name="xt")
                      for _ in range(n_dc)]
                for i in range(n_dc):
                    nc.sync.dma_start(xs[i][:], xT_dram[i * P:(i + 1) * P, :])
                for g in range(n_cc):
                    for t in range(n_cols // NT):
                        ps = psump.tile([P, NT], dt.float32, tag="mm")
                        for i in range(n_dc):
                            nc.tensor.matmul(
                                ps[:],
                                lhsT=w_tiles[i][:, g * P:(g + 1) * P],
                                rhs=xs[i][:, t * NT:(t + 1) * NT],
                                start=(i == 0), stop=(i == n_dc - 1))
                        sink(g, t, ps, bias_tiles[g])

            def q_sink(g, t, ps, bias):
                nc.scalar.activation(qp_t[g][:, t * NT:(t + 1) * NT], ps[:],
                                     Act.Identity, bias=bias[:])
            project(qT, q_len, w_sb["wq"], b_sb["bq"], q_sink)

            def k_sink(g, t, ps, bias):
                dst = kcat_t[g][:, kv_len + t * NT:kv_len + (t + 1) * NT]
                nc.scalar.activation(dst, ps[:], Act.Identity, bias=bias[:])
            project(kT, kv_len, w_sb["wk"], b_sb["bk"], k_sink)
            for g in range(n_cc):
                nc.sync.dma_start(kpT_out[g * P:(g + 1) * P, :],
                                  kcat_t[g][:, kv_len:kvt])

            vp_t = [projp.tile([P, kv_len], dt.bfloat16, tag=f"vpt{g}", name=f"vpt{g}")
                    for g in range(n_cc)]

            def v_sink(g, t, ps, bias):
                nc.scalar.activation(vp_t[g][:, t * NT:(t + 1) * NT], ps[:],
                                     Act.Identity, bias=bias[:])
            project(vT, kv_len, w_sb["wv"], b_sb["bv"], v_sink)
            for g in range(n_cc):
                nc.sync.dma_start(vpT_out[g * P:(g + 1) * P, :], vp_t[g][:])

            # vp^T -> normal layout into vaug chunks [n_kvc/2 ..)
            for g in range(n_cc):
                for cb in range(kv_len // P):
                    pt = psumtp.tile([P, P], dt.bfloat16, tag="tr")
                    nc.tensor.transpose(pt[:], vp_t[g][:, cb * P:(cb + 1) * P],
                                        idn[:])
                    va = vaug[n_kvc // 2 + cb].rearrange(
                        "p (g x) -> p g x", x=HD + 1)
                    dst = va[:, g * 2:g * 2 + 2, 0:HD]
                    src = pt.rearrange("p (g x) -> p g x", x=HD)
                    nc.vector.tensor_copy(dst, src)

            # ---- attention ----
            for qb in range(n_qt):
                mts = [maskp.tile([P, NT], dt.bfloat16, tag="mask", name="mask")
                       for _ in range(n_kvc)]
                for i in range(n_kvc):
                    nc.sync.dma_start(mts[i][:], maskt[qb, i])
                for h in range(heads_per_core):
                    g, po_ = h // 2, (h % 2) * HD
                    po = psumop.tile([HD + 1, NT], dt.float32, tag="po")
                    for i in range(n_kvc):
                        ps = psump.tile([P, NT], dt.float32, tag="mm")
                        nc.tensor.matmul(
                            ps[:],
                            lhsT=kcat_t[g][po_:po_ + HD, i * P:(i + 1) * P],
                            rhs=qp_t[g][po_:po_ + HD, qb * NT:(qb + 1) * NT],
                            start=True, stop=True)
                        pe = attnp.tile([P, NT], dt.bfloat16, tag="pexp", bufs=3)
                        nc.scalar.activation(pe[:], ps[:], Act.Exp)
                        nc.vector.tensor_mul(pe[:], pe[:], mts[i][:])
                        nc.tensor.matmul(
                            po[:],
                            lhsT=vaug[i][:, h * (HD + 1):(h + 1) * (HD + 1)],
                            rhs=pe[:],
                            start=(i == 0), stop=(i == n_kvc - 1))
                    rec = attnp.tile([1, NT], dt.float32, tag="rec", bufs=2)
                    nc.vector.reciprocal(rec[:], po[HD:HD + 1, :])
                    prb = psump.tile([HD, NT], dt.float32, tag="prb", bufs=1)
                    nc.tensor.matmul(prb[:], lhsT=ones_hd[:], rhs=rec[:],
                                     start=True, stop=True)
                    rcb = attnp.tile([HD, NT], dt.float32, tag="rcb", bufs=2)
                    nc.scalar.activation(rcb[:], prb[:], Act.Copy)
                    nc.vector.tensor_mul(
                        out_t[g][po_:po_ + HD, qb * NT:(qb + 1) * NT],
                        po[0:HD, :], rcb[:])

            # ---- output projection + ReduceScatter ----
            rs_in = dramp.tile([d, q_len], dt.bfloat16, tag="rsin")
            rs_out = dramp.tile([d // GROUPS, q_len], dt.bfloat16, tag="rsout")
            for dc in range(n_dc):
                for t in range(n_qt):
                    px = psump.tile([P, NT], dt.float32, tag="mm")
                    for cc in range(n_cc):
                        nc.tensor.matmul(
                            px[:],
                            lhsT=wl_sb[cc][:, dc * P:(dc + 1) * P],
                            rhs=out_t[cc][:, t * NT:(t + 1) * NT],
                            start=(cc == 0), stop=(cc == n_cc - 1))
                    xs = attnp.tile([P, NT], dt.bfloat16, tag="xpart", bufs=3)
                    nc.vector.tensor_copy(xs[:], px[:])
                    nc.sync.dma_start(
                        rs_in[dc * P:(dc + 1) * P, t * NT:(t + 1) * NT], xs[:])
            nc.gpsimd.collective_compute(
                "ReduceScatter", Alu.add, replica_groups=rg,
                ins=[rs_in.opt()], outs=[rs_out.opt()])
            for g in range(n_cc):
                for t in range(n_qt):
                    xf = attnp.tile([P, NT], dt.bfloat16, tag="xfin", bufs=3)
                    nc.sync.dma_start(
                        xf[:], rs_out[g * P:(g + 1) * P, t * NT:(t + 1) * NT])
                    xo = attnp.tile([P, NT], dt.float32, tag="xout", bufs=3)
                    nc.scalar.activation(xo[:], xf[:], Act.Identity,
                                         bias=b_sb["bl"][g][:])
                    nc.sync.dma_start(
                        xT_out[g * P:(g + 1) * P, t * NT:(t + 1) * NT], xo[:])

    nc.compile()
    return nc


def make_in_maps(q, k, v, past_k, past_v, mask, Wq, bq, Wk, bk, Wv, bv, Wl, bl):
    """Host-side sharding/layout prep. Returns list of 8 dicts."""
    f32 = np.float32
    scale = 1.0 / np.sqrt(HD)
    Ql, KVl = q.shape[1], k.shape[1]
    KVTl = 2 * KVl
    qT = [np.ascontiguousarray(np.asarray(q[b], f32).T).astype(BF16) for b in range(B)]
    kT = [np.ascontiguousarray(np.asarray(k[b], f32).T).astype(BF16) for b in range(B)]
    vT = [np.ascontiguousarray(np.asarray(v[b], f32).T).astype(BF16) for b in range(B)]
    pkT = [np.ascontiguousarray(np.asarray(past_k[b], f32).T).astype(BF16) for b in range(B)]
    pvb = [np.asarray(past_v[b], f32).astype(BF16) for b in range(B)]
    # (1 - mask)^T tiled to (n_qt, n_kvc, P, NT)
    m01t = []
    for b in range(B):
        mt = (1.0 - np.asarray(mask[b], f32)).T  # (KVT, Q)
        mt = mt.reshape(KVTl // P, P, Ql // NT, NT).transpose(2, 0, 1, 3)
        m01t.append(np.ascontiguousarray(mt).astype(BF16))
    WqT = (np.asarray(Wq, f32).T * scale).astype(BF16)
    WkT = np.asarray(Wk, f32).T.astype(BF16)
    WvT = np.asarray(Wv, f32).T.astype(BF16)
    WlT = np.asarray(Wl, f32).T.astype(BF16)
    bq_s = (np.asarray(bq, f32) * scale).reshape(-1, 1)
    bk_s = np.asarray(bk, f32).reshape(-1, 1)
    bv_s = np.asarray(bv, f32).reshape(-1, 1)
    bl_s = np.asarray(bl, f32).reshape(-1, 1)
    eye = np.eye(P, dtype=f32).astype(BF16)

    in_maps = []
    for c in range(N_CORES):
        b, hg = c // GROUPS, c % GROUPS
        cs = slice(C * hg, C * (hg + 1))
        in_maps.append({
            "qT": qT[b], "kT": kT[b], "vT": vT[b],
            "pkT": np.ascontiguousarray(pkT[b][cs]),
            "pv": np.ascontiguousarray(pvb[b][:, cs]),
            "maskt": m01t[b],
            "wqT": np.ascontiguousarray(WqT[:, cs]),
            "wkT": np.ascontiguousarray(WkT[:, cs]),
            "wvT": np.ascontiguousarray(WvT[:, cs]),
            "wlT": np.ascontiguousarray(WlT[cs, :]),
            "bq": np.ascontiguousarray(bq_s[cs]),
            "bk": np.ascontiguousarray(bk_s[cs]),
            "bv": np.ascontiguousarray(bv_s[cs]),
            "bl": np.ascontiguousarray(bl_s[cs]),
            "ident": eye,
        })
    return in_maps


def assemble(results, past_k, past_v):
    """Gather per-core outputs into full (x, k_cat, v_cat)."""
    f32 = np.float32
    Ql, KVl = results[0]["xT"].shape[1], past_k.shape[1]
    x = np.empty((B, Ql, D), f32)
    kp = np.empty((B, KVl, D), f32)
    vp = np.empty((B, KVl, D), f32)
    for c in range(N_CORES):
        b, hg = c // GROUPS, c % GROUPS
        cs = slice(C * hg, C * (hg + 1))
        x[b][:, cs] = np.asarray(results[c]["xT"], f32).T
        kp[b][:, cs] = np.asarray(results[c]["kpT"], f32).T
        vp[b][:, cs] = np.asarray(results[c]["vpT"], f32).T
    k_cat = np.concatenate([np.asarray(past_k, f32), kp], axis=1)
    v_cat = np.concatenate([np.asarray(past_v, f32), vp], axis=1)
    return x, k_cat, v_cat


_NC_CACHE = {}


def _get_nc():
    if "nc" not in _NC_CACHE:
        _NC_CACHE["nc"] = build_graph()
    return _NC_CACHE["nc"]


def _ensure_ntff_hook():
    """Register the axon NTFF profile hook if the antenv shim is absent."""
    try:
        from antenv.axon_hooks import get_axon_ntff_profile_hook  # noqa: F401
        return
    except ImportError:
        pass
    import types
    import antenv
    if "/root/.axon_site" not in sys.path:
        sys.path.insert(0, "/root/.axon_site")
    from trn_agent_boot.trn_boot import _ntff_profile_via_ctypes
    mod = types.ModuleType("antenv.axon_hooks")
    _h = {"h": _ntff_profile_via_ctypes("/opt/axon/libaxon_pjrt.so")}
    mod.get_axon_ntff_profile_hook = lambda: _h["h"]
    mod.set_axon_ntff_profile_hook = lambda h: _h.__setitem__("h", h)
    sys.modules["antenv.axon_hooks"] = mod
    antenv.axon_hooks = mod


def run(trace=False, **inputs):
    from concourse import bass_utils
    if trace:
        _ensure_ntff_hook()
        bass_utils.upload_artifacts = lambda tmpdir: tmpdir
    nc = _get_nc()
    in_maps = make_in_maps(**inputs)
    res = bass_utils.run_bass_kernel_spmd(
        nc, in_maps, core_ids=list(range(N_CORES)), trace=trace)
    outs = assemble(res.results, inputs["past_k"], inputs["past_v"])
    return outs, res


def kernel(**inputs):
    outs, _ = run(trace=False, **inputs)
    return outs


# revision 16
# speedup vs baseline: 1.0530x; 1.0530x over previous
"""Distributed Bass kernel for nn_AttentionLayer (B=2, Q=2048, KV=2048, D=1024, H=16).

Sharding: 8 cores = 2 batches x 4 head-groups (4 heads / 256 dims each).
Per core:
  qp^T = (Wq^T/8 slice) @ q^T       (bias folded, 1/sqrt(64) folded)
  kcat^T = [past_k^T slice | kp^T]  (kp^T = Wk^T slice @ k^T)
  v_aug  = normal-layout v_cat slice with a ones column per head
  S^T(kv,q) = kh^T.T @ qh^T ; P^T = exp(S^T) * (1-mask)^T
  out^T(65,q) = v_aug.T @ P^T       (row 64 = softmax denominators)
  x^T partial = Wl^T slice.T @ out_all^T  -> ReduceScatter(group of 4) -> +bl
Host does layout prep (transpose/cast/scale) and final gather/assembly.
"""

import os
import sys
import numpy as np

for _p in ("/opt/trn_rl_repo",):
    if _p not in sys.path:
        sys.path.insert(0, _p)

import ml_dtypes

BF16 = ml_dtypes.bfloat16

# ---- problem constants (hardcoded per spec) ----
B, Q, KV, D, H = 2, 2048, 2048, 1024, 16
HD = D // H           # 64
N_CORES = 8
GROUPS = 4            # head groups (= cores per batch)
HPC = H // GROUPS     # heads per core = 4
C = HPC * HD          # head-cols per core = 256
KVT = 2 * KV          # concat kv length = 4096
P = 128               # partitions
NT = 512              # matmul free-dim tile


def build_graph(q_len=Q, kv_len=KV, d=D, heads_per_core=HPC):
    """Build the SPMD Bass graph (same program on all 8 cores)."""
    import concourse.bass as bass
    import concourse.bacc as bacc
    import concourse.tile as tile
    from concourse import mybir

    dt = mybir.dt
    Act = mybir.ActivationFunctionType
    Alu = mybir.AluOpType

    c = heads_per_core * HD              # 256
    kvt = 2 * kv_len
    n_qt = q_len // NT                   # q free-dim tiles (4)
    n_kvc = kvt // P                     # kv partition chunks (32)
    n_dc = d // P                        # D chunks (8)
    n_cc = c // P                        # head-col chunks (2)

    nc = bacc.Bacc("TRN2", target_bir_lowering=False, debug=False,
                   num_devices=N_CORES)

    # ---- I/O ----
    qT = nc.dram_tensor("qT", [d, q_len], dt.bfloat16, kind="ExternalInput")
    kT = nc.dram_tensor("kT", [d, kv_len], dt.bfloat16, kind="ExternalInput")
    vT = nc.dram_tensor("vT", [d, kv_len], dt.bfloat16, kind="ExternalInput")
    pkT = nc.dram_tensor("pkT", [c, kv_len], dt.bfloat16, kind="ExternalInput")
    pv = nc.dram_tensor("pv", [kv_len, c], dt.bfloat16, kind="ExternalInput")
    maskt = nc.dram_tensor("maskt", [n_qt, n_kvc, P, NT], dt.bfloat16,
                           kind="ExternalInput")
    wqT = nc.dram_tensor("wqT", [d, c], dt.bfloat16, kind="ExternalInput")
    wkT = nc.dram_tensor("wkT", [d, c], dt.bfloat16, kind="ExternalInput")
    wvT = nc.dram_tensor("wvT", [d, c], dt.bfloat16, kind="ExternalInput")
    wlT = nc.dram_tensor("wlT", [c, d], dt.bfloat16, kind="ExternalInput")
    bq = nc.dram_tensor("bq", [c, 1], dt.float32, kind="ExternalInput")
    bk = nc.dram_tensor("bk", [c, 1], dt.float32, kind="ExternalInput")
    bv = nc.dram_tensor("bv", [c, 1], dt.float32, kind="ExternalInput")
    bl = nc.dram_tensor("bl", [d // GROUPS, 1], dt.float32, kind="ExternalInput")
    ident = nc.dram_tensor("ident", [P, P], dt.bfloat16, kind="ExternalInput")

    kpT_out = nc.dram_tensor("kpT", [c, kv_len], dt.bfloat16, kind="ExternalOutput")
    vpT_out = nc.dram_tensor("vpT", [c, kv_len], dt.bfloat16, kind="ExternalOutput")
    xT_out = nc.dram_tensor("xT", [d // GROUPS, q_len], dt.float32,
                            kind="ExternalOutput")

    rg = [[0, 1, 2, 3], [4, 5, 6, 7]]

    with tile.TileContext(nc) as tc:
        with (
            tc.tile_pool(name="const", bufs=1) as constp,
            tc.tile_pool(name="xt", bufs=10) as xtp,
            tc.tile_pool(name="proj", bufs=1) as projp,
            tc.tile_pool(name="vaug", bufs=1) as vaugp,
            tc.tile_pool(name="mask", bufs=2 * n_kvc + 4) as maskp,
            tc.tile_pool(name="attn", bufs=3) as attnp,
            tc.tile_pool(name="psum", bufs=3, space="PSUM") as psump,
            tc.tile_pool(name="psum_o", bufs=3, space="PSUM") as psumop,
            tc.tile_pool(name="psum_t", bufs=1, space="PSUM") as psumtp,
            tc.tile_pool(name="dram", bufs=1, space="DRAM") as dramp,
        ):
            # ---- constants ----
            idn = constp.tile([P, P], dt.bfloat16, tag="ident", name="ident_sb")
            nc.sync.dma_start(idn[:], ident[:])
            ones_hd = constp.tile([1, HD], dt.float32, tag="ones", name="ones_hd")
            nc.vector.memset(ones_hd[:], 1.0)
            w_sb = {}
            for name, w in (("wq", wqT), ("wk", wkT), ("wv", wvT)):
                tiles = []
                for i in range(n_dc):
                    t = constp.tile([P, c], dt.bfloat16, tag=f"{name}{i}", name=f"{name}{i}")
                    nc.sync.dma_start(t[:], w[i * P:(i + 1) * P, :])
                    tiles.append(t)
                w_sb[name] = tiles
            wl_sb = []
            for i in range(n_cc):
                t = constp.tile([P, d], dt.bfloat16, tag=f"wl{i}", name=f"wl{i}")
                nc.sync.dma_start(t[:], wlT[i * P:(i + 1) * P, :])
                wl_sb.append(t)
            b_sb = {}
            for name, bten in (("bq", bq), ("bk", bk), ("bv", bv), ("bl", bl)):
                tiles = []
                for i in range(n_cc):
                    t = constp.tile([P, 1], dt.float32, tag=f"{name}{i}", name=f"{name}{i}")
                    nc.sync.dma_start(t[:], bten[i * P:(i + 1) * P, :])
                    tiles.append(t)
                b_sb[name] = tiles

            # ---- persistent activations ----
            qp_t = [constp.tile([P, q_len], dt.bfloat16, tag=f"qpt{g}", name=f"qpt{g}")
                    for g in range(n_cc)]
            kcat_t = [constp.tile([P, kvt], dt.bfloat16, tag=f"kct{g}", name=f"kct{g}")
                      for g in range(n_cc)]
            out_t = [constp.tile([P, q_len], dt.bfloat16, tag=f"ot{g}", name=f"ot{g}")
                     for g in range(n_cc)]
            vaug = [vaugp.tile([P, heads_per_core * (HD + 1)], dt.bfloat16,
                               tag=f"va{i}", name=f"va{i}") for i in range(n_kvc)]

            # past_k^T -> left half of kcat_t
            for g in range(n_cc):
                nc.sync.dma_start(kcat_t[g][:, 0:kv_len], pkT[g * P:(g + 1) * P, :])
            # past_v -> vaug chunks [0 .. n_kvc/2), strided head groups + ones col
            for i in range(n_kvc // 2):
                va = vaug[i].rearrange("p (g x) -> p g x", x=HD + 1)
                src = pv[i * P:(i + 1) * P, :].rearrange("p (g x) -> p g x", x=HD)
                nc.sync.dma_start(va[:, :, 0:HD], src)
                nc.vector.memset(va[:, :, HD:HD + 1], 1.0)
            for i in range(n_kvc // 2, n_kvc):
                va = vaug[i].rearrange("p (g x) -> p g x", x=HD + 1)
                nc.vector.memset(va[:, :, HD:HD + 1], 1.0)

            # ---- projections (xt streamed in half-width chunks) ----
            def project(xT_dram, n_cols, w_tiles, bias_tiles, sink):
                """sink(g, t, psum_ap, bias) consumes each (128, NT) tile of
                (W^T slice @ x^T) = (c x n_cols), in ^T layout."""
                n_halves = 2 if n_cols >= 2 * NT else 1
                h2 = n_cols // n_halves
                for half in range(n_halves):
                    xs = [xtp.tile([P, h2], dt.bfloat16, tag="xt", name="xt")
                          for _ in range(n_dc)]
                    for i in range(n_dc):
                        nc.sync.dma_start(
                            xs[i][:],
                            xT_dram[i * P:(i + 1) * P, half * h2:(half + 1) * h2])
                    for g in range(n_cc):
                        for tl in range(h2 // NT):
                            t = half * (h2 // NT) + tl
                            ps = psump.tile([P, NT], dt.float32, tag="mm",
                                            name="ps")
                            for i in range(n_dc):
                                nc.tensor.matmul(
                                    ps[:],
                                    lhsT=w_tiles[i][:, g * P:(g + 1) * P],
                                    rhs=xs[i][:, tl * NT:(tl + 1) * NT],
                                    start=(i == 0), stop=(i == n_dc - 1))
                            sink(g, t, ps, bias_tiles[g])

            def q_sink(g, t, ps, bias):
                nc.scalar.activation(qp_t[g][:, t * NT:(t + 1) * NT], ps[:],
                                     Act.Identity, bias=bias[:])
            project(qT, q_len, w_sb["wq"], b_sb["bq"], q_sink)

            def k_sink(g, t, ps, bias):
                dst = kcat_t[g][:, kv_len + t * NT:kv_len + (t + 1) * NT]
                nc.scalar.activation(dst, ps[:], Act.Identity, bias=bias[:])
            project(kT, kv_len, w_sb["wk"], b_sb["bk"], k_sink)
            for g in range(n_cc):
                nc.sync.dma_start(kpT_out[g * P:(g + 1) * P, :],
                                  kcat_t[g][:, kv_len:kvt])

            vp_t = [projp.tile([P, kv_len], dt.bfloat16, tag=f"vpt{g}", name=f"vpt{g}")
                    for g in range(n_cc)]

            def v_sink(g, t, ps, bias):
                nc.scalar.activation(vp_t[g][:, t * NT:(t + 1) * NT], ps[:],
                                     Act.Identity, bias=bias[:])
            project(vT, kv_len, w_sb["wv"], b_sb["bv"], v_sink)
            for g in range(n_cc):
                nc.sync.dma_start(vpT_out[g * P:(g + 1) * P, :], vp_t[g][:])

            # vp^T -> normal layout into vaug chunks [n_kvc/2 ..)
            for g in range(n_cc):
                for cb in range(kv_len // P):
                    pt = psumtp.tile([P, P], dt.bfloat16, tag="tr", name="pt")
                    nc.tensor.transpose(pt[:], vp_t[g][:, cb * P:(cb + 1) * P],
                                        idn[:])
                    va = vaug[n_kvc // 2 + cb].rearrange(
                        "p (g x) -> p g x", x=HD + 1)
                    dst = va[:, g * 2:g * 2 + 2, 0:HD]
                    src = pt.rearrange("p (g x) -> p g x", x=HD)
                    nc.vector.tensor_copy(dst, src)

            # ---- attention (qb pairs share stationary operands) ----
            for qs in range(0, n_qt, 2):
                qbs = tuple(range(qs, min(qs + 2, n_qt)))
                mts = {}
                for j, qb in enumerate(qbs):
                    for i in range(n_kvc):
                        m = maskp.tile([P, NT], dt.bfloat16, tag="mask",
                                       name="mask")
                        nc.sync.dma_start(m[:], maskt[qb, i])
                        mts[(i, j)] = m
                for h in range(heads_per_core):
                    g, po_ = h // 2, (h % 2) * HD
                    po = [psumop.tile([HD + 1, NT], dt.float32, tag="po",
                                      name="po") for _ in qbs]
                    for i in range(n_kvc):
                        pes = []
                        for j, qb in enumerate(qbs):
                            ps = psump.tile([P, NT], dt.float32, tag="mm",
                                            name="ps")
                            nc.tensor.matmul(
                                ps[:],
                                lhsT=kcat_t[g][po_:po_ + HD, i * P:(i + 1) * P],
                                rhs=qp_t[g][po_:po_ + HD, qb * NT:(qb + 1) * NT],
                                start=True, stop=True)
                            pe = attnp.tile([P, NT], dt.bfloat16, tag="pexp",
                                            bufs=6, name="pe")
                            nc.scalar.activation(pe[:], ps[:], Act.Exp)
                            eng = nc.gpsimd if (i + j) % 3 == 0 else nc.vector
                            eng.tensor_mul(pe[:], pe[:], mts[(i, j)][:])
                            pes.append(pe)
                        for j in range(len(qbs)):
                            nc.tensor.matmul(
                                po[j][:],
                                lhsT=vaug[i][:, h * (HD + 1):(h + 1) * (HD + 1)],
                                rhs=pes[j][:],
                                start=(i == 0), stop=(i == n_kvc - 1))
                    for j, qb in enumerate(qbs):
                        rec = attnp.tile([1, NT], dt.float32, tag="rec", bufs=2,
                                         name="rec")
                        nc.vector.reciprocal(rec[:], po[j][HD:HD + 1, :])
                        prb = psump.tile([HD, NT], dt.float32, tag="prb", bufs=1,
                                         name="prb")
                        nc.tensor.matmul(prb[:], lhsT=ones_hd[:], rhs=rec[:],
                                         start=True, stop=True)
                        rcb = attnp.tile([HD, NT], dt.float32, tag="rcb", bufs=2,
                                         name="rcb")
                        nc.vector.tensor_copy(rcb[:], prb[:])
                        nc.vector.tensor_mul(
                            out_t[g][po_:po_ + HD, qb * NT:(qb + 1) * NT],
                            po[j][0:HD, :], rcb[:])

            # ---- output projection + split ReduceScatter ----
            rs_in = dramp.tile([d, q_len], dt.bfloat16, tag="rsin", name="rsin")
            rs_out = [dramp.tile([d // GROUPS // 2, q_len], dt.bfloat16,
                                 tag=f"rsout{s}", name=f"rsout{s}")
                      for s in range(2)]
            for s in range(2):
                for dc in range(s * n_dc // 2, (s + 1) * n_dc // 2):
                    for t in range(n_qt):
                        px = psump.tile([P, NT], dt.float32, tag="mm", name="px")
                        for cc in range(n_cc):
                            nc.tensor.matmul(
                                px[:],
                                lhsT=wl_sb[cc][:, dc * P:(dc + 1) * P],
                                rhs=out_t[cc][:, t * NT:(t + 1) * NT],
                                start=(cc == 0), stop=(cc == n_cc - 1))
                        xs = attnp.tile([P, NT], dt.bfloat16, tag="xpart",
                                        bufs=3, name="xs")
                        nc.vector.tensor_copy(xs[:], px[:])
                        nc.sync.dma_start(
                            rs_in[dc * P:(dc + 1) * P, t * NT:(t + 1) * NT],
                            xs[:])
                half = d // 2
                nc.gpsimd.collective_compute(
                    "ReduceScatter", Alu.add, replica_groups=rg,
                    ins=[rs_in[s * half:(s + 1) * half, :].opt()],
                    outs=[rs_out[s].opt()])
            for s in range(2):
                for t in range(n_qt):
                    xf = attnp.tile([P, NT], dt.bfloat16, tag="xfin", bufs=3,
                                    name="xf")
                    nc.sync.dma_start(
                        xf[:], rs_out[s][:, t * NT:(t + 1) * NT])
                    xo = attnp.tile([P, NT], dt.float32, tag="xout", bufs=3,
                                    name="xo")
                    nc.scalar.activation(xo[:], xf[:], Act.Identity,
                                         bias=b_sb["bl"][s][:])
                    nc.sync.dma_start(
                        xT_out[s * P:(s + 1) * P, t * NT:(t + 1) * NT], xo[:])

    nc.compile()
    return nc


def make_in_maps(q, k, v, past_k, past_v, mask, Wq, bq, Wk, bk, Wv, bv, Wl, bl):
    """Host-side sharding/layout prep. Returns list of 8 dicts."""
    f32 = np.float32
    scale = 1.0 / np.sqrt(HD)
    Ql, KVl = q.shape[1], k.shape[1]
    KVTl = 2 * KVl
    qT = [np.ascontiguousarray(np.asarray(q[b], f32).T).astype(BF16) for b in range(B)]
    kT = [np.ascontiguousarray(np.asarray(k[b], f32).T).astype(BF16) for b in range(B)]
    vT = [np.ascontiguousarray(np.asarray(v[b], f32).T).astype(BF16) for b in range(B)]
    pkT = [np.ascontiguousarray(np.asarray(past_k[b], f32).T).astype(BF16) for b in range(B)]
    pvb = [np.asarray(past_v[b], f32).astype(BF16) for b in range(B)]
    # (1 - mask)^T tiled to (n_qt, n_kvc, P, NT)
    m01t = []
    for b in range(B):
        mt = (1.0 - np.asarray(mask[b], f32)).T  # (KVT, Q)
        mt = mt.reshape(KVTl // P, P, Ql // NT, NT).transpose(2, 0, 1, 3)
        m01t.append(np.ascontiguousarray(mt).astype(BF16))
    WqT = (np.asarray(Wq, f32).T * scale).astype(BF16)
    WkT = np.asarray(Wk, f32).T.astype(BF16)
    WvT = np.asarray(Wv, f32).T.astype(BF16)
    WlT = np.asarray(Wl, f32).T.astype(BF16)
    bq_s = (np.asarray(bq, f32) * scale).reshape(-1, 1)
    bk_s = np.asarray(bk, f32).reshape(-1, 1)
    bv_s = np.asarray(bv, f32).reshape(-1, 1)
    bl_s = np.asarray(bl, f32).reshape(-1, 1)
    eye = np.eye(P, dtype=f32).astype(BF16)

    in_maps = []
    for c in range(N_CORES):
        b, hg = c // GROUPS, c % GROUPS
        cs = slice(C * hg, C * (hg + 1))
        in_maps.append({
            "qT": qT[b], "kT": kT[b], "vT": vT[b],
            "pkT": np.ascontiguousarray(pkT[b][cs]),
            "pv": np.ascontiguousarray(pvb[b][:, cs]),
            "maskt": m01t[b],
            "wqT": np.ascontiguousarray(WqT[:, cs]),
            "wkT": np.ascontiguousarray(WkT[:, cs]),
            "wvT": np.ascontiguousarray(WvT[:, cs]),
            "wlT": np.ascontiguousarray(WlT[cs, :]),
            "bq": np.ascontiguousarray(bq_s[cs]),
            "bk": np.ascontiguousarray(bk_s[cs]),
            "bv": np.ascontiguousarray(bv_s[cs]),
            "bl": np.ascontiguousarray(np.concatenate(
                [bl_s[128 * hg:128 * hg + 128],
                 bl_s[D // 2 + 128 * hg:D // 2 + 128 * hg + 128]])),
            "ident": eye,
        })
    return in_maps


def assemble(results, past_k, past_v):
    """Gather per-core outputs into full (x, k_cat, v_cat)."""
    f32 = np.float32
    Ql, KVl = results[0]["xT"].shape[1], past_k.shape[1]
    x = np.empty((B, Ql, D), f32)
    kp = np.empty((B, KVl, D), f32)
    vp = np.empty((B, KVl, D), f32)
    for c in range(N_CORES):
        b, hg = c // GROUPS, c % GROUPS
        cs = slice(C * hg, C * (hg + 1))
        xt = np.asarray(results[c]["xT"], f32)
        x[b][:, 128 * hg:128 * hg + 128] = xt[0:128].T
        x[b][:, D // 2 + 128 * hg:D // 2 + 128 * hg + 128] = xt[128:256].T
        kp[b][:, cs] = np.asarray(results[c]["kpT"], f32).T
        vp[b][:, cs] = np.asarray(results[c]["vpT"], f32).T
    k_cat = np.concatenate([np.asarray(past_k, f32), kp], axis=1)
    v_cat = np.concatenate([np.asarray(past_v, f32), vp], axis=1)
    return x, k_cat, v_cat


_NC_CACHE = {}


def _get_nc():
    if "nc" not in _NC_CACHE:
        _NC_CACHE["nc"] = build_graph()
    return _NC_CACHE["nc"]


def _ensure_ntff_hook():
    """Register the axon NTFF profile hook if the antenv shim is absent."""
    try:
        from antenv.axon_hooks import get_axon_ntff_profile_hook  # noqa: F401
        return
    except ImportError:
        pass
    import types
    import antenv
    if "/root/.axon_site" not in sys.path:
        sys.path.insert(0, "/root/.axon_site")
    from trn_agent_boot.trn_boot import _ntff_profile_via_ctypes
    mod = types.ModuleType("antenv.axon_hooks")
    _h = {"h": _ntff_profile_via_ctypes("/opt/axon/libaxon_pjrt.so")}
    mod.get_axon_ntff_profile_hook = lambda: _h["h"]
    mod.set_axon_ntff_profile_hook = lambda h: _h.__setitem__("h", h)
    sys.modules["antenv.axon_hooks"] = mod
    antenv.axon_hooks = mod


def run(trace=False, **inputs):
    from concourse import bass_utils
    if trace:
        _ensure_ntff_hook()
        bass_utils.upload_artifacts = lambda tmpdir: tmpdir
    nc = _get_nc()
    in_maps = make_in_maps(**inputs)
    res = bass_utils.run_bass_kernel_spmd(
        nc, in_maps, core_ids=list(range(N_CORES)), trace=trace)
    outs = assemble(res.results, inputs["past_k"], inputs["past_v"])
    return outs, res


def kernel(**inputs):
    outs, _ = run(trace=False, **inputs)
    return outs
